# revision 35
# baseline (speedup 1.0000x reference)
"""T5-style causal multi-head attention (B=4, S=2048, E=1024, H=16, D=64)
on 8 NeuronCores. Sharding: core c handles batch c//2 and head half c%2
(8 heads). Host sums the two row-parallel partial output projections per
batch.

v6: host pre-transposes x to [E, S] bf16 (no on-device transposes).
Far tiles (block distance >=2) use the saturated bucket-31 bias as an
ACT per-partition bias; near tiles multiply an exp(bias) table (mask
folds in as x0) on DVE. Truncated matmul widths on the causal diagonal.
Per-qc fused pipeline; the previous chunk's output-projection blocks are
interleaved into the attention loop to keep the PE fed while ACT drains.
Softmax denominator via ones-column in V, batched reciprocal_approx_fast
+ selector-matmul broadcast.
"""
import sys

sys.path.insert(0, "/opt/trn_rl_repo")

import numpy as np
import ml_dtypes

import concourse.bass as bass
import concourse.mybir as mybir
import concourse.tile as tile
from concourse import bacc
from concourse.bass_utils import run_bass_kernel_spmd

F32, BF16 = mybir.dt.float32, mybir.dt.bfloat16
AF = mybir.ActivationFunctionType
MULT = mybir.AluOpType.mult

B, S, E, H, D = 4, 2048, 1024, 16, 64
HL = H // 2          # heads per core
HD = HL * D          # 512, per-core head dims
NUM_BUCKETS, MAX_DISTANCE = 32, 128
NT = S // 128        # 16 token blocks
NE = E // 128        # 8 embed chunks
NMI = 5              # near-tile m in {-3..1}, mi = m+3 in {0..4}

_NC_CACHE = {}

# head-selector matrix for the denominator broadcast matmul:
# head h's reciprocal row lives at partition 32*(h%4) of den-slot h//4;
# sel[:, hp*128+p] picks head (2hp + p//64) for output partition p.
_SEL = np.zeros((128, 512), dtype=ml_dtypes.bfloat16)
for _hp in range(4):
    for _j in range(2):
        _h = 2 * _hp + _j
        _SEL[32 * (_h % 4), _hp * 128 + 64 * _j:_hp * 128 + 64 * (_j + 1)] = 1.0


def _off(mi):
    """Valid-q offset within a 512-wide score tile at block index mi."""
    return 128 * max(0, 3 - mi)


# ---------------------------------------------------------------- host side

def _np_bucket(distance):
    """Mirror reference._relative_position_bucket for causal (distance>=0),
    float32 arithmetic like jnp."""
    max_exact = NUM_BUCKETS // 2  # 16
    is_small = distance < max_exact
    safe = np.maximum(distance, 1).astype(np.float32)
    log_scale = np.log(safe / np.float32(max_exact)).astype(np.float32) / np.float32(
        np.log(np.float32(MAX_DISTANCE / max_exact))
    )
    large = max_exact + (log_scale * np.float32(NUM_BUCKETS - max_exact)).astype(
        np.int32
    )
    large = np.minimum(large, NUM_BUCKETS - 1)
    return np.where(is_small, distance, large)


def _build_etab(rel_bias_half):
    """rel_bias_half [8, 32] -> etab [4 hp, 128 k, 5 mi, 2 h, 512 q] bf16
    holding exp(bias) with causal mask as 0.  mi = (4*qc - kb) + 3 in 0..4
    (near tiles only; all farther tiles saturate to bucket 31)."""
    qq = np.arange(512)[None, :]
    kk = np.arange(128)[:, None]
    tiles = []
    for mi in range(NMI):
        m = mi - 3
        dd = 128 * m + qq - kk  # q - k distance, [128, 512]
        bucket = _np_bucket(np.maximum(dd, 0))
        vals = np.exp(rel_bias_half[:, bucket].astype(np.float32))  # [8,128,512]
        vals = np.where(dd[None] >= 0, vals, np.float32(0.0))
        tiles.append(vals.astype(np.float32))
    t = np.stack(tiles, axis=0)  # [5 mi, 8 h, 128 k, 512 q]
    t = t.reshape(NMI, 4, 2, 128, 512).transpose(1, 3, 0, 2, 4)
    return np.ascontiguousarray(t).astype(ml_dtypes.bfloat16)


# -------------------------------------------------------------- device side

def _build_nc():
    nc = bacc.Bacc(None, target_bir_lowering=False)
    xqt_d = nc.dram_tensor("xqt", [E, S], BF16, kind="ExternalInput")
    xkvt_d = nc.dram_tensor("xkvt", [E, S], BF16, kind="ExternalInput")
    wq_d = nc.dram_tensor("wq", [E, HD], BF16, kind="ExternalInput")
    wk_d = nc.dram_tensor("wk", [E, HD], BF16, kind="ExternalInput")
    wv_d = nc.dram_tensor("wv", [E, HD], BF16, kind="ExternalInput")
    wo_d = nc.dram_tensor("wo", [HD, E], BF16, kind="ExternalInput")
    etab_d = nc.dram_tensor("etab", [4, 128, NMI, 2, 512], BF16,
                            kind="ExternalInput")
    b31_d = nc.dram_tensor("b31", [128, HL], F32, kind="ExternalInput")
    sel_d = nc.dram_tensor("sel", [128, 512], BF16, kind="ExternalInput")
    out_d = nc.dram_tensor("out", [S, E], F32, kind="ExternalOutput")

    with tile.TileContext(nc) as tc:
        with (
            tc.tile_pool(name="const", bufs=1) as pconst,
            tc.tile_pool(name="persist", bufs=1) as pper,
        ):
            sel_sb = pconst.tile([128, 4, 128], BF16)
            nc.sync.dma_start(
                out=sel_sb, in_=sel_d[:].rearrange("p (g n) -> p g n", n=128))

            qT = pper.tile([128, 4, S], BF16)         # [pair-dims, hp, tok]
            kT = pper.tile([128, 4, S], BF16)
            vA = pper.tile([128, NT, HL * 65], BF16)  # v + ones col per head
            oT = pper.tile([128, 4, S], BF16)
            wq_sb = pper.tile([128, NE, HD], BF16)
            wk_sb = pper.tile([128, NE, HD], BF16)
            wv_sb = pper.tile([128, NE, HD], BF16)
            wo_sb = pper.tile([128, 4, E], BF16)
            etab_sb = pper.tile([128, 4, NMI, 2, 512], BF16)
            b31_sb = pper.tile([128, HL], F32)

            vAr = vA.rearrange("p t (h c) -> p t h c", c=65)
            nc.vector.memset(vAr[:, :, :, 64:65], 1.0)

            def load_xt(x_d, quad, pool, tag):
                """DMA x^T tile [128, NE, 512] for one 512-token quad."""
                xT = pool.tile([128, NE, 512], BF16, tag=tag)
                for e in range(NE):
                    nc.sync.dma_start(
                        out=xT[:, e, :],
                        in_=x_d[e * 128:(e + 1) * 128,
                                quad * 512:(quad + 1) * 512])
                return xT

            # ---------------- phase K: k/v projections over xkv
            with (
                tc.tile_pool(name="kxt", bufs=2) as pkx,
                tc.tile_pool(name="kpsP", bufs=4, space="PSUM") as psP,
            ):
                for quad in range(4):
                    xT = load_xt(xkvt_d, quad, pkx, "xTk")
                    # stagger the remaining input DMAs behind early rows
                    if quad == 0:
                        for w_sb, w_dr in ((wk_sb, wk_d), (wv_sb, wv_d)):
                            nc.sync.dma_start(
                                out=w_sb,
                                in_=w_dr[:].rearrange("(e p) n -> p e n", p=128))
                    elif quad == 1:
                        nc.sync.dma_start(
                            out=wq_sb,
                            in_=wq_d[:].rearrange("(e p) n -> p e n", p=128))
                        nc.sync.dma_start(out=b31_sb, in_=b31_d[:])
                    elif quad == 2:
                        for hp in range(4):
                            nc.sync.dma_start(
                                out=etab_sb[:, hp], in_=etab_d[hp])
                    elif quad == 3:
                        nc.sync.dma_start(
                            out=wo_sb,
                            in_=wo_d[:].rearrange("(g p) n -> p g n", p=128))
                    for hc in range(4):
                        kps = psP.tile([128, 512], F32, tag="kps")
                        for e in range(NE):
                            nc.tensor.matmul(
                                kps, wk_sb[:, e, hc * 128:(hc + 1) * 128],
                                xT[:, e, :],
                                start=(e == 0), stop=(e == NE - 1))
                        nc.scalar.copy(
                            kT[:, hc, quad * 512:(quad + 1) * 512], kps)
                    for j in range(4):
                        vps = psP.tile([128, HD], F32, tag="kps")
                        for e in range(NE):
                            nc.tensor.matmul(
                                vps, xT[:, e, j * 128:(j + 1) * 128],
                                wv_sb[:, e, :],
                                start=(e == 0), stop=(e == NE - 1))
                        nc.scalar.copy(
                            vAr[:, quad * 4 + j, :, 0:64],
                            vps.rearrange("p (h c) -> p h c", c=64))

            # ---------------- per-qc: q-proj, deferred norm+out, attention
            with (
                tc.tile_pool(name="qxt", bufs=2) as pqx,
                tc.tile_pool(name="pP", bufs=4) as pP,
                tc.tile_pool(name="rec", bufs=2) as prec,
                tc.tile_pool(name="oev", bufs=3) as poev,
                tc.tile_pool(name="psS", bufs=2, space="PSUM") as psS,
                tc.tile_pool(name="psO", bufs=2, space="PSUM") as psO,
                tc.tile_pool(name="psX", bufs=2, space="PSUM") as psX,
            ):
                def normalize_half(qc, den2, half):
                    """Normalize head pairs 2*half, 2*half+1 (den slot half)."""
                    rec2 = prec.tile([128, 512], F32, tag="rec2")
                    recb2 = prec.tile([128, 512], BF16, tag="recb2")
                    nc.vector.reciprocal_approx_fast(rec2, den2[:, half, :])
                    nc.vector.tensor_copy(recb2, rec2)
                    for hp in (2 * half, 2 * half + 1):
                        rep = psX.tile([128, 512], F32, tag="x")
                        nc.tensor.matmul(
                            rep, sel_sb[:, hp, :], recb2,
                            start=True, stop=True)
                        repb = prec.tile([128, 512], BF16, tag="repb")
                        nc.vector.tensor_copy(repb, rep)
                        for hh in range(2):
                            sl_ = oT[hh * 64:(hh + 1) * 64, hp,
                                     qc * 512:(qc + 1) * 512]
                            nc.gpsimd.tensor_tensor(
                                out=sl_, in0=sl_,
                                in1=repb[hh * 64:(hh + 1) * 64, :], op=MULT)

                def out_proj_t(t, last=False):
                    oev = poev.tile([128, E], F32, tag="oev")
                    for ec in range(2):
                        ops = psX.tile([128, 512], F32, tag="x")
                        for hp in range(4):
                            nc.tensor.matmul(
                                ops, oT[:, hp, t * 128:(t + 1) * 128],
                                wo_sb[:, hp, ec * 512:(ec + 1) * 512],
                                start=(hp == 0), stop=(hp == 3))
                        drain = nc.scalar.copy if last else \
                            nc.vector.tensor_copy
                        drain(oev[:, ec * 512:(ec + 1) * 512], ops)
                    nc.sync.dma_start(
                        out=out_d[t * 128:(t + 1) * 128, :], in_=oev)

                prev = None  # (qc, den2) awaiting normalize + out-proj
                for qc in range(4):
                    # q projection for this 512-token chunk
                    xT = load_xt(xqt_d, qc, pqx, "xTq")
                    for hc in range(4):
                        qps = psX.tile([128, 512], F32, tag="x")
                        for e in range(NE):
                            nc.tensor.matmul(
                                qps, wq_sb[:, e, hc * 128:(hc + 1) * 128],
                                xT[:, e, :],
                                start=(e == 0), stop=(e == NE - 1))
                        nc.vector.tensor_copy(
                            qT[:, hc, qc * 512:(qc + 1) * 512], qps)

                    if prev is not None:
                        normalize_half(prev[0], prev[1], 0)
                        normalize_half(prev[0], prev[1], 1)

                    # attention for all head pairs at this qc; one deferred
                    # output-projection block per hp keeps the PE fed
                    nkb = 4 * qc + 4
                    den2 = prec.tile([128, 2, 512], F32, tag="den2")
                    # unused rows must stay finite: the selector matmul
                    # contracts all 128 partitions and 0*inf would NaN
                    nc.gpsimd.memset(den2, 1.0)
                    for hp in range(4):
                        h0, h1 = 2 * hp, 2 * hp + 1
                        o0 = psO.tile([65, 512], F32, tag="o")
                        o1 = psO.tile([65, 512], F32, tag="o")
                        for kb0 in range(0, nkb, 2):
                            mi_a = 4 * qc - kb0 + 3
                            mi_b = mi_a - 1
                            offs = (_off(mi_a), _off(mi_b))
                            s0 = psS.tile([128, 2, 512], F32, tag="s")
                            s1 = psS.tile([128, 2, 512], F32, tag="s")
                            p0 = pP.tile([128, 2, 512], BF16, tag="p")
                            p1 = pP.tile([128, 2, 512], BF16, tag="p")
                            for sl, of in ((0, offs[0]), (1, offs[1])):
                                kb = kb0 + sl
                                nc.tensor.matmul(
                                    s0[:, sl, of:],
                                    kT[0:64, hp, kb * 128:(kb + 1) * 128],
                                    qT[0:64, hp, qc * 512 + of:(qc + 1) * 512],
                                    start=True, stop=True)
                                nc.tensor.matmul(
                                    s1[:, sl, of:],
                                    kT[64:128, hp, kb * 128:(kb + 1) * 128],
                                    qT[64:128, hp,
                                       qc * 512 + of:(qc + 1) * 512],
                                    start=True, stop=True)
                            for s_, p_, h2 in ((s0, p0, 0), (s1, p1, 1)):
                                h = 2 * hp + h2
                                if mi_b >= NMI:      # both far
                                    nc.scalar.activation(
                                        p_, s_, AF.Exp, scale=0.125,
                                        bias=b31_sb[:, h:h + 1])
                                elif mi_a < NMI:     # both near
                                    nc.scalar.activation(
                                        p_, s_, AF.Exp, scale=0.125)
                                else:                # mixed far/near
                                    nc.scalar.activation(
                                        p_[:, 0, :], s_[:, 0, :], AF.Exp,
                                        scale=0.125, bias=b31_sb[:, h:h + 1])
                                    nc.scalar.activation(
                                        p_[:, 1, :], s_[:, 1, :], AF.Exp,
                                        scale=0.125)
                                for sl, mi in ((0, mi_a), (1, mi_b)):
                                    if mi < NMI:
                                        of = _off(mi)
                                        eng = nc.vector if sl == 0 else \
                                            nc.gpsimd
                                        eng.tensor_tensor(
                                            out=p_[:, sl, of:],
                                            in0=p_[:, sl, of:],
                                            in1=etab_sb[:, hp, mi, h2, of:],
                                            op=MULT)
                            for sl, of in ((0, offs[0]), (1, offs[1])):
                                kb = kb0 + sl
                                nc.tensor.matmul(
                                    o0[:, of:], vA[:, kb, h0 * 65:(h0 + 1) * 65],
                                    p0[:, sl, of:],
                                    start=(kb == 0), stop=(kb == nkb - 1))
                                nc.tensor.matmul(
                                    o1[:, of:], vA[:, kb, h1 * 65:(h1 + 1) * 65],
                                    p1[:, sl, of:],
                                    start=(kb == 0), stop=(kb == nkb - 1))
                        # drain raw O + denominator rows; PSUM freed fast
                        for hh, ops_o in ((0, o0), (1, o1)):
                            h = 2 * hp + hh
                            pr = 32 * (h % 4)
                            nc.vector.tensor_copy(
                                den2[pr:pr + 1, h // 4, :], ops_o[64:65, :])
                            nc.vector.tensor_copy(
                                oT[hh * 64:(hh + 1) * 64, hp,
                                   qc * 512:(qc + 1) * 512], ops_o[0:64, :])
                        # last chunk: normalize eagerly to shrink the tail
                        if qc == 3 and hp in (1, 3):
                            normalize_half(qc, den2, hp // 2)
                        if prev is not None:
                            out_proj_t(prev[0] * 4 + hp)
                    prev = (qc, den2)

                for t in range(12, 16):
                    out_proj_t(t, last=True)

    nc.compile()
    return nc


def _get_nc():
    if "nc" not in _NC_CACHE:
        _NC_CACHE["nc"] = _build_nc()
    return _NC_CACHE["nc"]


def _make_in_maps(np_inputs):
    bf = ml_dtypes.bfloat16
    inputs_q = np.asarray(np_inputs["inputs_q"], dtype=np.float32).astype(bf)
    inputs_kv = np.asarray(np_inputs["inputs_kv"], dtype=np.float32).astype(bf)
    Wq = np.asarray(np_inputs["Wq"], dtype=np.float32).astype(bf)
    Wk = np.asarray(np_inputs["Wk"], dtype=np.float32).astype(bf)
    Wv = np.asarray(np_inputs["Wv"], dtype=np.float32).astype(bf)
    Wo = np.asarray(np_inputs["Wo"], dtype=np.float32).astype(bf)
    rel_bias = np.asarray(np_inputs["rel_bias"], dtype=np.float32)

    in_maps = []
    for c in range(8):
        b, half = c // 2, c % 2
        sl = slice(half * HD, (half + 1) * HD)
        rb = rel_bias[half * HL:(half + 1) * HL]
        in_maps.append({
            "xqt": np.ascontiguousarray(inputs_q[b].T),
            "xkvt": np.ascontiguousarray(inputs_kv[b].T),
            "wq": np.ascontiguousarray(Wq[:, sl]),
            "wk": np.ascontiguousarray(Wk[:, sl]),
            "wv": np.ascontiguousarray(Wv[:, sl]),
            "wo": np.ascontiguousarray(Wo[sl, :]),
            "etab": _build_etab(rb),
            "b31": np.ascontiguousarray(
                np.tile(rb[:, 31][None, :], (128, 1)).astype(np.float32)),
            "sel": _SEL,
        })
    return in_maps


def kernel(inputs_q, inputs_kv, mask, Wq, Wk, Wv, Wo, rel_bias):
    nc = _get_nc()
    in_maps = _make_in_maps({
        "inputs_q": inputs_q, "inputs_kv": inputs_kv, "Wq": Wq, "Wk": Wk,
        "Wv": Wv, "Wo": Wo, "rel_bias": rel_bias})
    res = run_bass_kernel_spmd(nc, in_maps, core_ids=list(range(8)))
    out = np.stack(
        [res.results[2 * b]["out"] + res.results[2 * b + 1]["out"]
         for b in range(B)])
    return out.astype(np.float32)


# revision 39
# speedup vs baseline: 1.1666x; 1.1666x over previous
"""T5-style causal multi-head attention (B=4, S=2048, E=1024, H=16, D=64)
on 8 NeuronCores. Sharding: core c handles batch c//2 and head half c%2
(8 heads). Host sums the two row-parallel partial output projections per
batch.

v6: host pre-transposes x to [E, S] bf16 (no on-device transposes).
Far tiles (block distance >=2) use the saturated bucket-31 bias as an
ACT per-partition bias; near tiles multiply an exp(bias) table (mask
folds in as x0) on DVE. Truncated matmul widths on the causal diagonal.
Per-qc fused pipeline; the previous chunk's output-projection blocks are
interleaved into the attention loop to keep the PE fed while ACT drains.
Softmax denominator via ones-column in V, batched reciprocal_approx_fast
+ selector-matmul broadcast.
"""
import sys

sys.path.insert(0, "/opt/trn_rl_repo")

import numpy as np
import ml_dtypes

import concourse.bass as bass
import concourse.mybir as mybir
import concourse.tile as tile
from concourse import bacc
from concourse.bass_utils import run_bass_kernel_spmd

F32, BF16 = mybir.dt.float32, mybir.dt.bfloat16
AF = mybir.ActivationFunctionType
MULT = mybir.AluOpType.mult

B, S, E, H, D = 4, 2048, 1024, 16, 64
HL = H // 2          # heads per core
HD = HL * D          # 512, per-core head dims
NUM_BUCKETS, MAX_DISTANCE = 32, 128
NT = S // 128        # 16 token blocks
NE = E // 128        # 8 embed chunks
NMI = 5              # near-tile m in {-3..1}, mi = m+3 in {0..4}

_NC_CACHE = {}

# head-selector matrix for the denominator broadcast matmul:
# head h's reciprocal row lives at partition 32*(h%4) of den-slot h//4;
# sel[:, hp*128+p] picks head (2hp + p//64) for output partition p.
_SEL = np.zeros((128, 512), dtype=ml_dtypes.bfloat16)
for _hp in range(4):
    for _j in range(2):
        _h = 2 * _hp + _j
        _SEL[32 * (_h % 4), _hp * 128 + 64 * _j:_hp * 128 + 64 * (_j + 1)] = 1.0


def _off(mi):
    """Valid-q offset within a 512-wide score tile at block index mi."""
    return 128 * max(0, 3 - mi)


# ---------------------------------------------------------------- host side

def _np_bucket(distance):
    """Mirror reference._relative_position_bucket for causal (distance>=0),
    float32 arithmetic like jnp."""
    max_exact = NUM_BUCKETS // 2  # 16
    is_small = distance < max_exact
    safe = np.maximum(distance, 1).astype(np.float32)
    log_scale = np.log(safe / np.float32(max_exact)).astype(np.float32) / np.float32(
        np.log(np.float32(MAX_DISTANCE / max_exact))
    )
    large = max_exact + (log_scale * np.float32(NUM_BUCKETS - max_exact)).astype(
        np.int32
    )
    large = np.minimum(large, NUM_BUCKETS - 1)
    return np.where(is_small, distance, large)


def _build_etab(rel_bias_half):
    """rel_bias_half [8, 32] -> etab [4 hp, 128 k, 5 mi, 2 h, 512 q] bf16
    holding exp(bias) with causal mask as 0.  mi = (4*qc - kb) + 3 in 0..4
    (near tiles only; all farther tiles saturate to bucket 31)."""
    qq = np.arange(512)[None, :]
    kk = np.arange(128)[:, None]
    tiles = []
    for mi in range(NMI):
        m = mi - 3
        dd = 128 * m + qq - kk  # q - k distance, [128, 512]
        bucket = _np_bucket(np.maximum(dd, 0))
        vals = np.exp(rel_bias_half[:, bucket].astype(np.float32))  # [8,128,512]
        vals = np.where(dd[None] >= 0, vals, np.float32(0.0))
        tiles.append(vals.astype(np.float32))
    t = np.stack(tiles, axis=0)  # [5 mi, 8 h, 128 k, 512 q]
    t = t.reshape(NMI, 4, 2, 128, 512).transpose(1, 3, 0, 2, 4)
    return np.ascontiguousarray(t).astype(ml_dtypes.bfloat16)


# -------------------------------------------------------------- device side

def _build_nc():
    nc = bacc.Bacc(None, target_bir_lowering=False)
    xqt_d = nc.dram_tensor("xqt", [E, S], BF16, kind="ExternalInput")
    xkvt_d = nc.dram_tensor("xkvt", [E, S], BF16, kind="ExternalInput")
    wq_d = nc.dram_tensor("wq", [E, HD], BF16, kind="ExternalInput")
    wk_d = nc.dram_tensor("wk", [E, HD], BF16, kind="ExternalInput")
    wv_d = nc.dram_tensor("wv", [E, HD], BF16, kind="ExternalInput")
    wo_d = nc.dram_tensor("wo", [HD, E], BF16, kind="ExternalInput")
    etab_d = nc.dram_tensor("etab", [4, 128, NMI, 2, 512], BF16,
                            kind="ExternalInput")
    b31_d = nc.dram_tensor("b31", [128, HL], F32, kind="ExternalInput")
    sel_d = nc.dram_tensor("sel", [128, 512], BF16, kind="ExternalInput")
    out_d = nc.dram_tensor("out", [S, E], F32, kind="ExternalOutput")

    with tile.TileContext(nc) as tc:
        with (
            tc.tile_pool(name="const", bufs=1) as pconst,
            tc.tile_pool(name="persist", bufs=1) as pper,
        ):
            sel_sb = pconst.tile([128, 4, 128], BF16)
            nc.sync.dma_start(
                out=sel_sb, in_=sel_d[:].rearrange("p (g n) -> p g n", n=128))

            qT = pper.tile([128, 4, S], BF16)         # [pair-dims, hp, tok]
            kT = pper.tile([128, 4, S], BF16)
            vA = pper.tile([128, NT, HL * 65], BF16)  # v + ones col per head
            oT = pper.tile([128, 4, S], BF16)
            wq_sb = pper.tile([128, NE, HD], BF16)
            wk_sb = pper.tile([128, NE, HD], BF16)
            wv_sb = pper.tile([128, NE, HD], BF16)
            wo_sb = pper.tile([128, 4, E], BF16)
            etab_sb = pper.tile([128, 4, NMI, 2, 512], BF16)
            b31_sb = pper.tile([128, HL], F32)

            vAr = vA.rearrange("p t (h c) -> p t h c", c=65)
            nc.vector.memset(vAr[:, :, :, 64:65], 1.0)

            def load_xt(x_d, quad, pool, tag):
                """DMA x^T tile [128, NE, 512] for one 512-token quad."""
                xT = pool.tile([128, NE, 512], BF16, tag=tag)
                for e in range(NE):
                    nc.sync.dma_start(
                        out=xT[:, e, :],
                        in_=x_d[e * 128:(e + 1) * 128,
                                quad * 512:(quad + 1) * 512])
                return xT

            # ------- fused loop: kv-proj(quad), q-proj(quad), attn(quad)
            with (
                tc.tile_pool(name="kxt", bufs=2) as pkx,
                tc.tile_pool(name="qxt", bufs=2) as pqx,
                tc.tile_pool(name="pP", bufs=4) as pP,
                tc.tile_pool(name="rec", bufs=2) as prec,
                tc.tile_pool(name="oev", bufs=3) as poev,
                tc.tile_pool(name="psS", bufs=2, space="PSUM") as psS,
                tc.tile_pool(name="psO", bufs=2, space="PSUM") as psO,
                tc.tile_pool(name="psX", bufs=2, space="PSUM") as psX,
            ):
                def normalize_half(qc, den2, half):
                    """Normalize head pairs 2*half, 2*half+1 (den slot half)."""
                    rec2 = prec.tile([128, 512], F32, tag="rec2")
                    recb2 = prec.tile([128, 512], BF16, tag="recb2")
                    nc.vector.reciprocal_approx_fast(rec2, den2[:, half, :])
                    nc.vector.tensor_copy(recb2, rec2)
                    for hp in (2 * half, 2 * half + 1):
                        rep = psX.tile([128, 512], F32, tag="x")
                        nc.tensor.matmul(
                            rep, sel_sb[:, hp, :], recb2,
                            start=True, stop=True)
                        repb = prec.tile([128, 512], BF16, tag="repb")
                        nc.vector.tensor_copy(repb, rep)
                        for hh in range(2):
                            sl_ = oT[hh * 64:(hh + 1) * 64, hp,
                                     qc * 512:(qc + 1) * 512]
                            nc.vector.tensor_tensor(
                                out=sl_, in0=sl_,
                                in1=repb[hh * 64:(hh + 1) * 64, :], op=MULT)

                def out_proj_t(t, last=False):
                    oev = poev.tile([128, E], F32, tag="oev")
                    for ec in range(2):
                        ops = psX.tile([128, 512], F32, tag="x")
                        for hp in range(4):
                            nc.tensor.matmul(
                                ops, oT[:, hp, t * 128:(t + 1) * 128],
                                wo_sb[:, hp, ec * 512:(ec + 1) * 512],
                                start=(hp == 0), stop=(hp == 3))
                        drain = nc.scalar.copy if last else \
                            nc.vector.tensor_copy
                        drain(oev[:, ec * 512:(ec + 1) * 512], ops)
                    nc.sync.dma_start(
                        out=out_d[t * 128:(t + 1) * 128, :], in_=oev)

                prev = None  # (qc, den2) awaiting normalize + out-proj
                for qc in range(4):
                    # k/v projection for this 512-token quad
                    xTk = load_xt(xkvt_d, qc, pkx, "xTk")
                    if qc == 0:
                        for w_sb, w_dr in ((wk_sb, wk_d), (wv_sb, wv_d)):
                            nc.sync.dma_start(
                                out=w_sb,
                                in_=w_dr[:].rearrange("(e p) n -> p e n", p=128))
                    xTq = load_xt(xqt_d, qc, pqx, "xTq")
                    if qc == 0:
                        nc.sync.dma_start(
                            out=wq_sb,
                            in_=wq_d[:].rearrange("(e p) n -> p e n", p=128))
                        for hp in range(4):
                            nc.sync.dma_start(
                                out=etab_sb[:, hp], in_=etab_d[hp])
                        nc.sync.dma_start(out=b31_sb, in_=b31_d[:])
                    elif qc == 1:
                        nc.sync.dma_start(
                            out=wo_sb,
                            in_=wo_d[:].rearrange("(g p) n -> p g n", p=128))
                    for hc in range(4):
                        kps = psX.tile([128, 512], F32, tag="x")
                        for e in range(NE):
                            nc.tensor.matmul(
                                kps, wk_sb[:, e, hc * 128:(hc + 1) * 128],
                                xTk[:, e, :],
                                start=(e == 0), stop=(e == NE - 1))
                        nc.scalar.copy(
                            kT[:, hc, qc * 512:(qc + 1) * 512], kps)
                    for j in range(4):
                        vps = psX.tile([128, HD], F32, tag="x")
                        for e in range(NE):
                            nc.tensor.matmul(
                                vps, xTk[:, e, j * 128:(j + 1) * 128],
                                wv_sb[:, e, :],
                                start=(e == 0), stop=(e == NE - 1))
                        nc.scalar.copy(
                            vAr[:, qc * 4 + j, :, 0:64],
                            vps.rearrange("p (h c) -> p h c", c=64))

                    # q projection for this 512-token chunk
                    for hc in range(4):
                        qps = psX.tile([128, 512], F32, tag="x")
                        for e in range(NE):
                            nc.tensor.matmul(
                                qps, wq_sb[:, e, hc * 128:(hc + 1) * 128],
                                xTq[:, e, :],
                                start=(e == 0), stop=(e == NE - 1))
                        nc.vector.tensor_copy(
                            qT[:, hc, qc * 512:(qc + 1) * 512], qps)

                    if prev is not None:
                        normalize_half(prev[0], prev[1], 0)
                        normalize_half(prev[0], prev[1], 1)

                    # attention for all head pairs at this qc; one deferred
                    # output-projection block per hp keeps the PE fed
                    nkb = 4 * qc + 4
                    den2 = prec.tile([128, 2, 512], F32, tag="den2")
                    # unused rows must stay finite: the selector matmul
                    # contracts all 128 partitions and 0*inf would NaN
                    nc.gpsimd.memset(den2, 1.0)
                    for hp in range(4):
                        h0, h1 = 2 * hp, 2 * hp + 1
                        o0 = psO.tile([65, 512], F32, tag="o")
                        o1 = psO.tile([65, 512], F32, tag="o")
                        for kb0 in range(0, nkb, 2):
                            mi_a = 4 * qc - kb0 + 3
                            mi_b = mi_a - 1
                            offs = (_off(mi_a), _off(mi_b))
                            s0 = psS.tile([128, 2, 512], F32, tag="s")
                            s1 = psS.tile([128, 2, 512], F32, tag="s")
                            p0 = pP.tile([128, 2, 512], BF16, tag="p")
                            p1 = pP.tile([128, 2, 512], BF16, tag="p")
                            for sl, of in ((0, offs[0]), (1, offs[1])):
                                kb = kb0 + sl
                                nc.tensor.matmul(
                                    s0[:, sl, of:],
                                    kT[0:64, hp, kb * 128:(kb + 1) * 128],
                                    qT[0:64, hp, qc * 512 + of:(qc + 1) * 512],
                                    start=True, stop=True)
                                nc.tensor.matmul(
                                    s1[:, sl, of:],
                                    kT[64:128, hp, kb * 128:(kb + 1) * 128],
                                    qT[64:128, hp,
                                       qc * 512 + of:(qc + 1) * 512],
                                    start=True, stop=True)
                            for s_, p_, h2 in ((s0, p0, 0), (s1, p1, 1)):
                                h = 2 * hp + h2
                                if mi_b >= NMI:      # both far
                                    nc.scalar.activation(
                                        p_, s_, AF.Exp, scale=0.125,
                                        bias=b31_sb[:, h:h + 1])
                                elif mi_a < NMI:     # both near
                                    nc.scalar.activation(
                                        p_, s_, AF.Exp, scale=0.125)
                                else:                # mixed far/near
                                    nc.scalar.activation(
                                        p_[:, 0, :], s_[:, 0, :], AF.Exp,
                                        scale=0.125, bias=b31_sb[:, h:h + 1])
                                    nc.scalar.activation(
                                        p_[:, 1, :], s_[:, 1, :], AF.Exp,
                                        scale=0.125)
                                for sl, mi in ((0, mi_a), (1, mi_b)):
                                    if mi < NMI:
                                        of = _off(mi)
                                        nc.vector.tensor_tensor(
                                            out=p_[:, sl, of:],
                                            in0=p_[:, sl, of:],
                                            in1=etab_sb[:, hp, mi, h2, of:],
                                            op=MULT)
                            for sl, of in ((0, offs[0]), (1, offs[1])):
                                kb = kb0 + sl
                                nc.tensor.matmul(
                                    o0[:, of:], vA[:, kb, h0 * 65:(h0 + 1) * 65],
                                    p0[:, sl, of:],
                                    start=(kb == 0), stop=(kb == nkb - 1))
                                nc.tensor.matmul(
                                    o1[:, of:], vA[:, kb, h1 * 65:(h1 + 1) * 65],
                                    p1[:, sl, of:],
                                    start=(kb == 0), stop=(kb == nkb - 1))
                        # drain raw O + denominator rows; PSUM freed fast
                        for hh, ops_o in ((0, o0), (1, o1)):
                            h = 2 * hp + hh
                            pr = 32 * (h % 4)
                            nc.vector.tensor_copy(
                                den2[pr:pr + 1, h // 4, :], ops_o[64:65, :])
                            nc.vector.tensor_copy(
                                oT[hh * 64:(hh + 1) * 64, hp,
                                   qc * 512:(qc + 1) * 512], ops_o[0:64, :])
                        # last chunk: normalize eagerly to shrink the tail
                        if qc == 3 and hp in (1, 3):
                            normalize_half(qc, den2, hp // 2)
                        if prev is not None:
                            out_proj_t(prev[0] * 4 + hp)
                    prev = (qc, den2)

                for t in range(12, 16):
                    out_proj_t(t, last=True)

    nc.compile()
    return nc


def _get_nc():
    if "nc" not in _NC_CACHE:
        _NC_CACHE["nc"] = _build_nc()
    return _NC_CACHE["nc"]


def _make_in_maps(np_inputs):
    bf = ml_dtypes.bfloat16
    inputs_q = np.asarray(np_inputs["inputs_q"], dtype=np.float32).astype(bf)
    inputs_kv = np.asarray(np_inputs["inputs_kv"], dtype=np.float32).astype(bf)
    Wq = np.asarray(np_inputs["Wq"], dtype=np.float32).astype(bf)
    Wk = np.asarray(np_inputs["Wk"], dtype=np.float32).astype(bf)
    Wv = np.asarray(np_inputs["Wv"], dtype=np.float32).astype(bf)
    Wo = np.asarray(np_inputs["Wo"], dtype=np.float32).astype(bf)
    rel_bias = np.asarray(np_inputs["rel_bias"], dtype=np.float32)

    in_maps = []
    for c in range(8):
        b, half = c // 2, c % 2
        sl = slice(half * HD, (half + 1) * HD)
        rb = rel_bias[half * HL:(half + 1) * HL]
        in_maps.append({
            "xqt": np.ascontiguousarray(inputs_q[b].T),
            "xkvt": np.ascontiguousarray(inputs_kv[b].T),
            "wq": np.ascontiguousarray(Wq[:, sl]),
            "wk": np.ascontiguousarray(Wk[:, sl]),
            "wv": np.ascontiguousarray(Wv[:, sl]),
            "wo": np.ascontiguousarray(Wo[sl, :]),
            "etab": _build_etab(rb),
            "b31": np.ascontiguousarray(
                np.tile(rb[:, 31][None, :], (128, 1)).astype(np.float32)),
            "sel": _SEL,
        })
    return in_maps


def kernel(inputs_q, inputs_kv, mask, Wq, Wk, Wv, Wo, rel_bias):
    nc = _get_nc()
    in_maps = _make_in_maps({
        "inputs_q": inputs_q, "inputs_kv": inputs_kv, "Wq": Wq, "Wk": Wk,
        "Wv": Wv, "Wo": Wo, "rel_bias": rel_bias})
    res = run_bass_kernel_spmd(nc, in_maps, core_ids=list(range(8)))
    out = np.stack(
        [res.results[2 * b]["out"] + res.results[2 * b + 1]["out"]
         for b in range(B)])
    return out.astype(np.float32)
